# revision 5
# baseline (speedup 1.0000x reference)
"""TAGConv GNN on 8 trn2 cores — scatter-add rounds design.

Like the one-hot kernel, but segment-sum uses fp16 dma_scatter_add into a
DRAM accumulator. HW loses duplicate-index updates within one scatter call,
so edges are split into rounds (round r = r-th edge of each dst): indices
within a call are unique, and calls serialize on the accumulator via Tile
WAW deps (cross-call RMW accumulation was validated exact on HW).
"""
import os

os.environ.setdefault("JAX_COMPILATION_CACHE_DIR", "/tmp/jaxcache")
os.environ.setdefault("JAX_PERSISTENT_CACHE_MIN_COMPILE_TIME_SECS", "0")
os.environ.setdefault("JAX_PERSISTENT_CACHE_MIN_ENTRY_SIZE_BYTES", "0")

import numpy as np

import concourse.bass as bass
import concourse.bacc as bacc
import concourse.mybir as mybir
import concourse.tile as tile
from concourse import bass_utils

try:
    import jax

    jax.config.update("jax_compilation_cache_dir",
                      os.environ["JAX_COMPILATION_CACHE_DIR"])
    jax.config.update("jax_persistent_cache_min_compile_time_secs", 0.0)
    jax.config.update("jax_persistent_cache_min_entry_size_bytes", 0)
except Exception:
    pass

N, E, G = 50000, 800000, 128
F = 128
CLASSES = 10
HOPS, HLAYERS = 2, 2
NCORES = 8


def configure(n, e, csup=96):
    global N, E, PER, GRP, NPAD, NT, HALF, TRASH, CSUP
    N, E = n, e
    PER = N // NCORES
    GRP = (PER + 127) // 128
    NPAD = GRP * 128
    NT = NCORES * NPAD
    HALF = NT // 2
    TRASH = NPAD
    CSUP = csup


configure(N, E)

F16 = mybir.dt.float16
FP = mybir.dt.float32
I16 = mybir.dt.int16

NW = (HOPS + 1) * F + CLASSES


def _prep_edges(src, dst):
    """Rounds × halves cells; within a cell edges are src-sorted. Returns
    wrapped gather+scatter indices and per-cell chunk counts (SPMD-max)."""
    src = np.asarray(src).astype(np.int64)
    dst = np.asarray(dst).astype(np.int64)
    core = dst // PER
    local = dst - core * PER
    ps = (src // PER) * NPAD + (src % PER)
    hi = (ps >= HALF).astype(np.int64)

    # round = rank of the edge within its (global) dst
    o = np.argsort(dst, kind="stable")
    rank = np.empty(E, np.int64)
    ds = dst[o]
    starts = np.concatenate([[0], np.nonzero(np.diff(ds))[0] + 1])
    grpstart = np.zeros(E, np.int64)
    grpstart[starts] = starts
    grpstart = np.maximum.accumulate(grpstart)
    rank[o] = np.arange(E) - grpstart
    R = int(rank.max()) + 1

    key = (core * R + rank) * 2 + hi
    order = np.lexsort((ps, key))
    cnt = np.bincount(key, minlength=NCORES * R * 2).reshape(NCORES, R, 2)
    CH = (-(-cnt.max(axis=0) // 128)).astype(int)       # [R, 2] chunks per cell
    cells = []                                          # (c0, nch, is_b, r)
    c0 = 0
    for r in range(R):
        for h in range(2):
            if CH[r, h] == 0:
                continue
            cells.append((c0, int(CH[r, h]), h == 1, r))
            c0 += int(CH[r, h])
    NCH = c0
    TOT = NCH * 128

    choff = np.zeros((R, 2), int)
    for (c0s, nch, is_b, r) in cells:
        choff[r, 1 if is_b else 0] = c0s

    gidx = np.zeros((NCORES, TOT), np.int16)
    sidx = np.full((NCORES, TOT), TRASH, np.int16)
    sp = ps[order]
    sl = local[order]
    st = np.concatenate([[0], np.cumsum(cnt.reshape(-1))]).astype(int)
    for c in range(NCORES):
        for r in range(R):
            for h in range(2):
                k = (c * R + r) * 2 + h
                n = int(cnt[c, r, h])
                s0 = st[k]
                off = choff[r, h] * 128
                gidx[c, off : off + n] = (sp[s0 : s0 + n] - h * HALF).astype(np.int16)
                sidx[c, off : off + n] = sl[s0 : s0 + n].astype(np.int16)

    def wrap(a):
        return np.ascontiguousarray(a.reshape(NCORES, -1, 16).transpose(0, 2, 1))

    gs = np.concatenate([wrap(gidx), wrap(sidx)], axis=2)
    return gs, cells, NCH, TOT, R


def _build_program(cells, NCH, TOT, shared_tables=True):
    W16 = TOT // 16
    nc = bacc.Bacc("TRN2", target_bir_lowering=False, debug=False,
                   num_devices=NCORES)
    RG = [list(range(NCORES))]
    aspace = "Shared" if shared_tables else "Local"
    MAXCH = max(nch for (_, nch, _, _) in cells)

    BIGW = GRP * F
    big_d = nc.dram_tensor("big_h", [128, BIGW], F16, kind="ExternalInput")
    gs_d = nc.dram_tensor("gs_idx", [16, 2 * W16], I16, kind="ExternalInput")
    colf_d = nc.dram_tensor("colf", [128, 2 * GRP + 3 + CLASSES], FP,
                            kind="ExternalInput")
    wpk_d = nc.dram_tensor("wpk", [(HOPS + 1) * F, NW], F16, kind="ExternalInput")
    out_d = nc.dram_tensor("out", [G, CLASSES], FP, kind="ExternalOutput")

    mul = mybir.AluOpType.mult

    with tile.TileContext(nc) as tc:
        with (
            tc.tile_pool(name="const", bufs=1) as cp,
            tc.tile_pool(name="work", bufs=3) as wp,
            tc.tile_pool(name="psmm", bufs=3, space="PSUM") as pmm,
            tc.tile_pool(name="pstr", bufs=2, space="PSUM") as ptr,
            tc.tile_pool(name="psro", bufs=2, space="PSUM") as pro,
            tc.tile_pool(name="dram", bufs=1, space="DRAM") as dp,
        ):
            gidx_t = cp.tile([128, W16], I16)
            sidx_t = cp.tile([128, W16], I16)
            big_t = cp.tile([128, BIGW], F16)
            colf_t = cp.tile([128, 2 * GRP + 3 + CLASSES], FP)
            wall_t = cp.tile([128, HOPS + 1, NW], F16)
            normh_t = cp.tile([128, GRP], F16)
            nsq_t = cp.tile([128, GRP], F16)
            gsloth_t = cp.tile([128, GRP], F16)
            validh_t = cp.tile([128, GRP], F16)
            iotah_t = cp.tile([128, 128], F16)
            identh_t = cp.tile([128, 128], F16)
            og_t = cp.tile([128, GRP, 128], F16)
            tn_t = cp.tile([128, GRP, F], F16)
            agg_t = cp.tile([128, GRP, F], F16)
            fs_t = cp.tile([128, GRP, F], F16)
            f0T = cp.tile([128, NPAD], F16)
            f1T = cp.tile([128, NPAD], F16)
            f2T = cp.tile([128, NPAD], F16)
            ro2_t = cp.tile([128, F + 1], FP)
            cnt_t = cp.tile([128, 1], FP)
            rcp_t = cp.tile([128, 1], FP)
            hgh_t = cp.tile([128, F], F16)
            hgT_t = cp.tile([128, 128], F16)
            logit_t = cp.tile([128, CLASSES], FP)

            Ts = [dp.tile([NT, F], F16, addr_space=aspace, name=f"T{i}")
                  for i in range(2 * HLAYERS + HOPS)]
            ag_in = dp.tile([NPAD, F], F16)
            acc = dp.tile([NPAD + 128, F], F16)
            ar_in = dp.tile([128, F + 1], FP)
            ar_out = dp.tile([128, F + 1], FP)

            rearr = lambda ap: ap.rearrange("(g p) f -> p g f", p=128)

            for k in range(8):
                nc.sync.dma_start(gidx_t[16 * k : 16 * (k + 1), :], gs_d[:, 0:W16])
                nc.sync.dma_start(sidx_t[16 * k : 16 * (k + 1), :],
                                  gs_d[:, W16 : 2 * W16])
            nc.sync.dma_start(big_t[:], big_d[:, :])
            nc.sync.dma_start(colf_t[:], colf_d[:, :])
            for k in range(HOPS + 1):
                nc.sync.dma_start(wall_t[:, k, :], wpk_d[k * 128 : (k + 1) * 128, :])

            xt = big_t[:, 0 : GRP * F].rearrange("p (g f) -> p g f", f=F)
            normc = colf_t[:, 0:GRP]
            gslot = colf_t[:, GRP : 2 * GRP]
            b_t = colf_t[:, 2 * GRP : 2 * GRP + 3]
            bcr = colf_t[:, 2 * GRP + 3 : 2 * GRP + 3 + CLASSES]

            nc.gpsimd.iota(iotah_t[:], pattern=[[1, 128]], base=0,
                           channel_multiplier=0,
                           allow_small_or_imprecise_dtypes=True)
            icolh_t = cp.tile([128, 1], F16)
            nc.gpsimd.iota(icolh_t[:], pattern=[[0, 1]], base=0,
                           channel_multiplier=1,
                           allow_small_or_imprecise_dtypes=True)
            nc.vector.tensor_tensor(identh_t[:],
                                    icolh_t[:].broadcast_to([128, 128]),
                                    iotah_t[:], mybir.AluOpType.is_equal)
            nc.vector.tensor_copy(normh_t[:], normc)
            nc.vector.tensor_tensor(nsq_t[:], normh_t[:], normh_t[:], mul)
            nc.vector.tensor_copy(gsloth_t[:], gslot)
            nc.vector.tensor_scalar_add(validh_t[:], gsloth_t[:], 1.0)
            nc.vector.tensor_scalar_min(validh_t[:], validh_t[:], 1.0)
            nc.vector.tensor_tensor(
                og_t[:],
                gsloth_t[:].unsqueeze(2).broadcast_to([128, GRP, 128]),
                iotah_t[:].unsqueeze(1).broadcast_to([128, GRP, 128]),
                mybir.AluOpType.is_equal)

            def hop(Tsrc, fT, make_table):
                nc.vector.memset(agg_t[:], 0.0)
                nc.sync.dma_start(rearr(acc[0:NPAD, :]), agg_t[:])
                for (c0, nch, is_b, r) in cells:
                    vb = wp.tile([128, MAXCH, F], F16, name="vb", tag="vb")
                    srcv = Tsrc[HALF:, :] if is_b else Tsrc[:, :]
                    nc.gpsimd.dma_gather(
                        vb[:, 0:nch, :], srcv,
                        gidx_t[:, c0 * 8 : (c0 + nch) * 8],
                        nch * 128, nch * 128, F, single_packet=False)
                    nc.gpsimd.dma_scatter_add(
                        acc[:, :], vb[:, 0:nch, :],
                        sidx_t[:, c0 * 8 : (c0 + nch) * 8],
                        nch * 128, nch * 128, F, single_packet=False)
                nc.sync.dma_start(agg_t[:], rearr(acc[0:NPAD, :]))
                if make_table:
                    nc.vector.tensor_tensor(
                        tn_t[:], agg_t[:],
                        nsq_t[:].unsqueeze(2).broadcast_to([128, GRP, F]), mul)
                nc.vector.tensor_tensor(
                    fs_t[:], agg_t[:],
                    normh_t[:].unsqueeze(2).broadcast_to([128, GRP, F]), mul)
                for g in range(GRP):
                    gsl = slice(g * 128, (g + 1) * 128)
                    pt = ptr.tile([128, 128], F16, name="pt", tag="tr")
                    nc.tensor.transpose(pt[:], fs_t[:, g, :], identh_t[:])
                    nc.vector.tensor_copy(fT[:, gsl], pt[:])

            def emit_table(Tdst):
                nc.sync.dma_start(rearr(ag_in[:, :]), tn_t[:])
                nc.gpsimd.collective_compute(
                    "AllGather", mybir.AluOpType.bypass, replica_groups=RG,
                    ins=[ag_in[:, :].opt()], outs=[Tdst[:, :].opt()])

            # t0
            nc.vector.tensor_tensor(
                tn_t[:], xt,
                normh_t[:].unsqueeze(2).broadcast_to([128, GRP, F]), mul)
            emit_table(Ts[0])
            for g in range(GRP):
                gsl = slice(g * 128, (g + 1) * 128)
                pt = ptr.tile([128, 128], F16, name="pt0", tag="tr")
                nc.tensor.transpose(pt[:], xt[:, g, :], identh_t[:])
                nc.vector.tensor_copy(f0T[:, gsl], pt[:])

            pr = None
            for l in range(HLAYERS + 1):
                hop(Ts[2 * l], f1T, make_table=True)
                emit_table(Ts[2 * l + 1])
                hop(Ts[2 * l + 1], f2T, make_table=False)
                for g in range(GRP):
                    gsl = slice(g * 128, (g + 1) * 128)
                    ph = pmm.tile([128, 128], FP, name="ph", tag="mm")
                    for k, fk in enumerate((f0T, f1T, f2T)):
                        nc.tensor.matmul(ph[:], wall_t[:, k, l * F : (l + 1) * F],
                                         fk[:, gsl], start=(k == 0), stop=(k == 2))
                    nc.scalar.activation(f0T[:, gsl], ph[:],
                                         mybir.ActivationFunctionType.Relu,
                                         bias=b_t[:, l : l + 1])
                if l < HLAYERS:
                    for g in range(GRP):
                        gsl = slice(g * 128, (g + 1) * 128)
                        pt2 = ptr.tile([128, 128], F16, name="pt2", tag="tr")
                        nc.tensor.transpose(pt2[:], f0T[:, gsl], identh_t[:])
                        nc.vector.tensor_tensor(
                            tn_t[:, g, :], pt2[:],
                            normh_t[:, g : g + 1].broadcast_to([128, F]), mul)
                    emit_table(Ts[2 * l + 2])
                else:
                    pr = pro.tile([128, F + 1], FP, name="pr", tag="ro")
                    for g in range(GRP):
                        gsl = slice(g * 128, (g + 1) * 128)
                        pt3 = ptr.tile([128, 128], F16, name="pt3", tag="tr")
                        nc.tensor.transpose(pt3[:], f0T[:, gsl], identh_t[:])
                        rr = wp.tile([128, F + 1], F16, name="rr", tag="rr")
                        nc.vector.tensor_copy(rr[:, 0:F], pt3[:])
                        nc.vector.tensor_copy(rr[:, F : F + 1],
                                              validh_t[:, g : g + 1])
                        nc.tensor.matmul(pr[:], og_t[:, g, :], rr[:],
                                         start=(g == 0), stop=(g == GRP - 1))

            ro_t = cp.tile([128, F + 1], FP)
            nc.vector.tensor_copy(ro_t[:], pr[:])
            nc.sync.dma_start(ar_in[:, :], ro_t[:])
            nc.gpsimd.collective_compute(
                "AllReduce", mybir.AluOpType.add, replica_groups=RG,
                ins=[ar_in[:, :].opt()], outs=[ar_out[:, :].opt()])
            nc.sync.dma_start(ro2_t[:], ar_out[:, :])
            nc.vector.tensor_scalar_max(cnt_t[:], ro2_t[:, F : F + 1], 1.0)
            nc.vector.reciprocal(rcp_t[:], cnt_t[:])
            nc.vector.tensor_tensor(hgh_t[:], ro2_t[:, 0:F],
                                    rcp_t[:].broadcast_to([128, F]), mul)
            ptf = ptr.tile([128, 128], F16, name="ptf", tag="tr")
            nc.tensor.transpose(ptf[:], hgh_t[:], identh_t[:])
            nc.vector.tensor_copy(hgT_t[:], ptf[:])
            plog = pro.tile([128, CLASSES], FP, name="plog", tag="ro")
            nc.tensor.matmul(plog[:], hgT_t[:],
                             wall_t[:, 0, (HLAYERS + 1) * F :],
                             start=True, stop=True)
            nc.vector.tensor_tensor(logit_t[:], plog[:], bcr,
                                    mybir.AluOpType.add)
            nc.sync.dma_start(out_d[:, :], logit_t[:])

    nc.finalize()
    return nc


def _make_in_maps(x, src, dst, graph_ids, Ws, bs, Wc, bc):
    deg = np.bincount(np.asarray(dst).astype(np.int64), minlength=N)
    norm = np.where(deg < 1, 1.0, deg).astype(np.float32) ** -0.5
    gs, cells, NCH, TOT, R = _prep_edges(src, dst)

    x = np.asarray(x, np.float32)
    graph_ids = np.asarray(graph_ids, np.int64)
    wpk = np.zeros(((HOPS + 1) * F, NW), np.float16)
    for l in range(HLAYERS + 1):
        wpk[:, l * F : (l + 1) * F] = np.asarray(Ws[l], np.float32)
    wpk[0:F, (HLAYERS + 1) * F :] = np.asarray(Wc, np.float32)
    bcol = np.stack([np.asarray(b, np.float32) for b in bs], 1)
    bcr = np.tile(np.asarray(bc, np.float32)[None, :], (128, 1))

    in_maps = []
    for c in range(NCORES):
        xl = np.zeros((NPAD, F), np.float16)
        xl[:PER] = x[c * PER : (c + 1) * PER]
        big = xl.reshape(GRP, 128, F).transpose(1, 0, 2).reshape(128, GRP * F)
        nrm = np.ones(NPAD, np.float32)
        nrm[:PER] = norm[c * PER : (c + 1) * PER]
        gsl = np.full(NPAD, -1.0, np.float32)
        gsl[:PER] = graph_ids[c * PER : (c + 1) * PER]
        colf = np.zeros((128, 2 * GRP + 3 + CLASSES), np.float32)
        colf[:, 0:GRP] = nrm.reshape(GRP, 128).T
        colf[:, GRP : 2 * GRP] = gsl.reshape(GRP, 128).T
        colf[:, 2 * GRP : 2 * GRP + 3] = bcol
        colf[:, 2 * GRP + 3 :] = bcr
        in_maps.append(dict(big_h=np.ascontiguousarray(big), gs_idx=gs[c],
                            colf=colf, wpk=wpk))
    return in_maps, cells, NCH, TOT


def kernel(x, src, dst, graph_ids, W0, b0, W1, b1, W2, b2, Wc, bc, **_):
    in_maps, cells, NCH, TOT = _make_in_maps(
        x, src, dst, graph_ids, [W0, W1, W2], [b0, b1, b2], Wc, bc)
    nc = _build_program(cells, NCH, TOT)
    res = bass_utils.run_bass_kernel_spmd(nc, in_maps, core_ids=list(range(NCORES)))
    return np.asarray(res.results[0]["out"], np.float32)


# revision 6
# speedup vs baseline: 1.0049x; 1.0049x over previous
"""TAGConv GNN on 8 trn2 cores — scatter-add rounds design.

Like the one-hot kernel, but segment-sum uses fp16 dma_scatter_add into a
DRAM accumulator. HW loses duplicate-index updates within one scatter call,
so edges are split into rounds (round r = r-th edge of each dst): indices
within a call are unique, and calls serialize on the accumulator via Tile
WAW deps (cross-call RMW accumulation was validated exact on HW).
"""
import os

os.environ.setdefault("JAX_COMPILATION_CACHE_DIR", "/tmp/jaxcache")
os.environ.setdefault("JAX_PERSISTENT_CACHE_MIN_COMPILE_TIME_SECS", "0")
os.environ.setdefault("JAX_PERSISTENT_CACHE_MIN_ENTRY_SIZE_BYTES", "0")

import numpy as np

import concourse.bass as bass
import concourse.bacc as bacc
import concourse.mybir as mybir
import concourse.tile as tile
from concourse import bass_utils

try:
    import jax

    jax.config.update("jax_compilation_cache_dir",
                      os.environ["JAX_COMPILATION_CACHE_DIR"])
    jax.config.update("jax_persistent_cache_min_compile_time_secs", 0.0)
    jax.config.update("jax_persistent_cache_min_entry_size_bytes", 0)
except Exception:
    pass

N, E, G = 50000, 800000, 128
F = 128
CLASSES = 10
HOPS, HLAYERS = 2, 2
NCORES = 8


def configure(n, e, csup=96):
    global N, E, PER, GRP, NPAD, NT, HALF, TRASH, CSUP
    N, E = n, e
    PER = N // NCORES
    GRP = (PER + 127) // 128
    NPAD = GRP * 128
    NT = NCORES * NPAD
    HALF = NT // 2
    TRASH = NPAD
    CSUP = csup


configure(N, E)

F16 = mybir.dt.float16
FP = mybir.dt.float32
I16 = mybir.dt.int16

NW = (HOPS + 1) * F + CLASSES


def _prep_edges(src, dst):
    """Rounds × halves cells; within a cell edges are src-sorted. Returns
    wrapped gather+scatter indices and per-cell chunk counts (SPMD-max)."""
    src = np.asarray(src).astype(np.int64)
    dst = np.asarray(dst).astype(np.int64)
    core = dst // PER
    local = dst - core * PER
    ps = (src // PER) * NPAD + (src % PER)
    hi = (ps >= HALF).astype(np.int64)

    # round = rank of the edge within its (global) dst
    o = np.argsort(dst, kind="stable")
    rank = np.empty(E, np.int64)
    ds = dst[o]
    starts = np.concatenate([[0], np.nonzero(np.diff(ds))[0] + 1])
    grpstart = np.zeros(E, np.int64)
    grpstart[starts] = starts
    grpstart = np.maximum.accumulate(grpstart)
    rank[o] = np.arange(E) - grpstart
    R = int(rank.max()) + 1

    key = (core * R + rank) * 2 + hi
    order = np.lexsort((ps, key))
    cnt = np.bincount(key, minlength=NCORES * R * 2).reshape(NCORES, R, 2)
    CH = (-(-cnt.max(axis=0) // 128)).astype(int)       # [R, 2] chunks per cell
    cells = []                                          # (c0, nch, is_b, r)
    c0 = 0
    for r in range(R):
        for h in range(2):
            if CH[r, h] == 0:
                continue
            cells.append((c0, int(CH[r, h]), h == 1, r))
            c0 += int(CH[r, h])
    NCH = c0
    TOT = NCH * 128

    choff = np.zeros((R, 2), int)
    for (c0s, nch, is_b, r) in cells:
        choff[r, 1 if is_b else 0] = c0s

    gidx = np.zeros((NCORES, TOT), np.int16)
    sidx = np.full((NCORES, TOT), TRASH, np.int16)
    sp = ps[order]
    sl = local[order]
    st = np.concatenate([[0], np.cumsum(cnt.reshape(-1))]).astype(int)
    for c in range(NCORES):
        for r in range(R):
            for h in range(2):
                k = (c * R + r) * 2 + h
                n = int(cnt[c, r, h])
                s0 = st[k]
                off = choff[r, h] * 128
                gidx[c, off : off + n] = (sp[s0 : s0 + n] - h * HALF).astype(np.int16)
                sidx[c, off : off + n] = sl[s0 : s0 + n].astype(np.int16)

    def wrap(a):
        return np.ascontiguousarray(a.reshape(NCORES, -1, 16).transpose(0, 2, 1))

    gs = np.concatenate([wrap(gidx), wrap(sidx)], axis=2)
    return gs, cells, NCH, TOT, R


def _build_program(cells, NCH, TOT, shared_tables=True):
    W16 = TOT // 16
    nc = bacc.Bacc("TRN2", target_bir_lowering=False, debug=False,
                   num_devices=NCORES)
    RG = [list(range(NCORES))]
    aspace = "Shared" if shared_tables else "Local"
    MAXCH = max(nch for (_, nch, _, _) in cells)

    CW = 2 * GRP + 3 + CLASSES                  # colf carried in fp16
    BIGW = GRP * F + CW + (HOPS + 1) * NW       # x | colf-bits | folded wpk
    big_d = nc.dram_tensor("big_h", [128, BIGW], F16, kind="ExternalInput")
    gs_d = nc.dram_tensor("gs_idx", [16, 2 * W16], I16, kind="ExternalInput")
    out_d = nc.dram_tensor("out", [G, CLASSES], FP, kind="ExternalOutput")

    mul = mybir.AluOpType.mult

    with tile.TileContext(nc) as tc:
        with (
            tc.tile_pool(name="const", bufs=1) as cp,
            tc.tile_pool(name="work", bufs=3) as wp,
            tc.tile_pool(name="psmm", bufs=3, space="PSUM") as pmm,
            tc.tile_pool(name="pstr", bufs=2, space="PSUM") as ptr,
            tc.tile_pool(name="psro", bufs=2, space="PSUM") as pro,
            tc.tile_pool(name="dram", bufs=1, space="DRAM") as dp,
        ):
            gidx_t = cp.tile([128, W16], I16)
            sidx_t = cp.tile([128, W16], I16)
            big_t = cp.tile([128, BIGW], F16)
            nsq_t = cp.tile([128, GRP], F16)
            validh_t = cp.tile([128, GRP], F16)
            bt32_t = cp.tile([128, 3], FP)
            bcr32_t = cp.tile([128, CLASSES], FP)
            iotah_t = cp.tile([128, 128], F16)
            identh_t = cp.tile([128, 128], F16)
            og_t = cp.tile([128, GRP, 128], F16)
            tn_t = cp.tile([128, GRP, F], F16)
            agg_t = cp.tile([128, GRP, F], F16)
            fs_t = cp.tile([128, GRP, F], F16)
            f0T = cp.tile([128, NPAD], F16)
            f1T = cp.tile([128, NPAD], F16)
            f2T = cp.tile([128, NPAD], F16)
            ro2_t = cp.tile([128, F + 1], FP)
            cnt_t = cp.tile([128, 1], FP)
            rcp_t = cp.tile([128, 1], FP)
            hgh_t = cp.tile([128, F], F16)
            hgT_t = cp.tile([128, 128], F16)
            logit_t = cp.tile([128, CLASSES], FP)

            Ts = [dp.tile([NT, F], F16, addr_space=aspace, name=f"T{i}")
                  for i in range(2 * HLAYERS + HOPS)]
            ag_in = dp.tile([NPAD, F], F16)
            acc = dp.tile([NPAD + 128, F], F16)
            ar_in = dp.tile([128, F + 1], FP)
            ar_out = dp.tile([128, F + 1], FP)

            rearr = lambda ap: ap.rearrange("(g p) f -> p g f", p=128)

            for k in range(8):
                nc.sync.dma_start(gidx_t[16 * k : 16 * (k + 1), :], gs_d[:, 0:W16])
                nc.sync.dma_start(sidx_t[16 * k : 16 * (k + 1), :],
                                  gs_d[:, W16 : 2 * W16])
            nc.sync.dma_start(big_t[:], big_d[:, :])

            xt = big_t[:, 0 : GRP * F].rearrange("p (g f) -> p g f", f=F)
            colf_t = big_t[:, GRP * F : GRP * F + CW]
            wall_t = big_t[:, GRP * F + CW :].rearrange(
                "p (k w) -> p k w", w=NW)
            normh_t = colf_t[:, 0:GRP]
            gsloth_t = colf_t[:, GRP : 2 * GRP]
            b_t = bt32_t
            bcr = bcr32_t

            nc.gpsimd.iota(iotah_t[:], pattern=[[1, 128]], base=0,
                           channel_multiplier=0,
                           allow_small_or_imprecise_dtypes=True)
            icolh_t = cp.tile([128, 1], F16)
            nc.gpsimd.iota(icolh_t[:], pattern=[[0, 1]], base=0,
                           channel_multiplier=1,
                           allow_small_or_imprecise_dtypes=True)
            nc.vector.tensor_tensor(identh_t[:],
                                    icolh_t[:].broadcast_to([128, 128]),
                                    iotah_t[:], mybir.AluOpType.is_equal)
            nc.vector.tensor_copy(bt32_t[:], colf_t[:, 2 * GRP : 2 * GRP + 3])
            nc.vector.tensor_copy(bcr32_t[:],
                                  colf_t[:, 2 * GRP + 3 : 2 * GRP + 3 + CLASSES])
            nc.vector.tensor_tensor(nsq_t[:], normh_t[:], normh_t[:], mul)
            nc.vector.tensor_scalar_add(validh_t[:], gsloth_t[:], 1.0)
            nc.vector.tensor_scalar_min(validh_t[:], validh_t[:], 1.0)
            nc.vector.tensor_tensor(
                og_t[:],
                gsloth_t[:].unsqueeze(2).broadcast_to([128, GRP, 128]),
                iotah_t[:].unsqueeze(1).broadcast_to([128, GRP, 128]),
                mybir.AluOpType.is_equal)

            def hop(Tsrc, fT, make_table):
                nc.vector.memset(agg_t[:], 0.0)
                nc.sync.dma_start(rearr(acc[0:NPAD, :]), agg_t[:])
                for (c0, nch, is_b, r) in cells:
                    vb = wp.tile([128, MAXCH, F], F16, name="vb", tag="vb")
                    srcv = Tsrc[HALF:, :] if is_b else Tsrc[:, :]
                    nc.gpsimd.dma_gather(
                        vb[:, 0:nch, :], srcv,
                        gidx_t[:, c0 * 8 : (c0 + nch) * 8],
                        nch * 128, nch * 128, F, single_packet=False)
                    nc.gpsimd.dma_scatter_add(
                        acc[:, :], vb[:, 0:nch, :],
                        sidx_t[:, c0 * 8 : (c0 + nch) * 8],
                        nch * 128, nch * 128, F, single_packet=False)
                nc.sync.dma_start(agg_t[:], rearr(acc[0:NPAD, :]))
                if make_table:
                    nc.vector.tensor_tensor(
                        tn_t[:], agg_t[:],
                        nsq_t[:].unsqueeze(2).broadcast_to([128, GRP, F]), mul)
                nc.vector.tensor_tensor(
                    fs_t[:], agg_t[:],
                    normh_t[:].unsqueeze(2).broadcast_to([128, GRP, F]), mul)
                for g in range(GRP):
                    gsl = slice(g * 128, (g + 1) * 128)
                    pt = ptr.tile([128, 128], F16, name="pt", tag="tr")
                    nc.tensor.transpose(pt[:], fs_t[:, g, :], identh_t[:])
                    nc.vector.tensor_copy(fT[:, gsl], pt[:])

            def emit_table(Tdst):
                nc.sync.dma_start(rearr(ag_in[:, :]), tn_t[:])
                nc.gpsimd.collective_compute(
                    "AllGather", mybir.AluOpType.bypass, replica_groups=RG,
                    ins=[ag_in[:, :].opt()], outs=[Tdst[:, :].opt()])

            # t0
            nc.vector.tensor_tensor(
                tn_t[:], xt,
                normh_t[:].unsqueeze(2).broadcast_to([128, GRP, F]), mul)
            emit_table(Ts[0])
            for g in range(GRP):
                gsl = slice(g * 128, (g + 1) * 128)
                pt = ptr.tile([128, 128], F16, name="pt0", tag="tr")
                nc.tensor.transpose(pt[:], xt[:, g, :], identh_t[:])
                nc.vector.tensor_copy(f0T[:, gsl], pt[:])

            pr = None
            for l in range(HLAYERS + 1):
                hop(Ts[2 * l], f1T, make_table=True)
                emit_table(Ts[2 * l + 1])
                hop(Ts[2 * l + 1], f2T, make_table=False)
                for g in range(GRP):
                    gsl = slice(g * 128, (g + 1) * 128)
                    ph = pmm.tile([128, 128], FP, name="ph", tag="mm")
                    for k, fk in enumerate((f0T, f1T, f2T)):
                        nc.tensor.matmul(ph[:], wall_t[:, k, l * F : (l + 1) * F],
                                         fk[:, gsl], start=(k == 0), stop=(k == 2))
                    nc.scalar.activation(f0T[:, gsl], ph[:],
                                         mybir.ActivationFunctionType.Relu,
                                         bias=b_t[:, l : l + 1])
                if l < HLAYERS:
                    for g in range(GRP):
                        gsl = slice(g * 128, (g + 1) * 128)
                        pt2 = ptr.tile([128, 128], F16, name="pt2", tag="tr")
                        nc.tensor.transpose(pt2[:], f0T[:, gsl], identh_t[:])
                        nc.vector.tensor_tensor(
                            tn_t[:, g, :], pt2[:],
                            normh_t[:, g : g + 1].broadcast_to([128, F]), mul)
                    emit_table(Ts[2 * l + 2])
                else:
                    pr = pro.tile([128, F + 1], FP, name="pr", tag="ro")
                    for g in range(GRP):
                        gsl = slice(g * 128, (g + 1) * 128)
                        pt3 = ptr.tile([128, 128], F16, name="pt3", tag="tr")
                        nc.tensor.transpose(pt3[:], f0T[:, gsl], identh_t[:])
                        rr = wp.tile([128, F + 1], F16, name="rr", tag="rr")
                        nc.vector.tensor_copy(rr[:, 0:F], pt3[:])
                        nc.vector.tensor_copy(rr[:, F : F + 1],
                                              validh_t[:, g : g + 1])
                        nc.tensor.matmul(pr[:], og_t[:, g, :], rr[:],
                                         start=(g == 0), stop=(g == GRP - 1))

            ro_t = cp.tile([128, F + 1], FP)
            nc.vector.tensor_copy(ro_t[:], pr[:])
            nc.sync.dma_start(ar_in[:, :], ro_t[:])
            nc.gpsimd.collective_compute(
                "AllReduce", mybir.AluOpType.add, replica_groups=RG,
                ins=[ar_in[:, :].opt()], outs=[ar_out[:, :].opt()])
            nc.sync.dma_start(ro2_t[:], ar_out[:, :])
            nc.vector.tensor_scalar_max(cnt_t[:], ro2_t[:, F : F + 1], 1.0)
            nc.vector.reciprocal(rcp_t[:], cnt_t[:])
            nc.vector.tensor_tensor(hgh_t[:], ro2_t[:, 0:F],
                                    rcp_t[:].broadcast_to([128, F]), mul)
            ptf = ptr.tile([128, 128], F16, name="ptf", tag="tr")
            nc.tensor.transpose(ptf[:], hgh_t[:], identh_t[:])
            nc.vector.tensor_copy(hgT_t[:], ptf[:])
            plog = pro.tile([128, CLASSES], FP, name="plog", tag="ro")
            nc.tensor.matmul(plog[:], hgT_t[:],
                             wall_t[:, 0, (HLAYERS + 1) * F :],
                             start=True, stop=True)
            nc.vector.tensor_tensor(logit_t[:], plog[:], bcr,
                                    mybir.AluOpType.add)
            nc.sync.dma_start(out_d[:, :], logit_t[:])

    nc.finalize()
    return nc


def _make_in_maps(x, src, dst, graph_ids, Ws, bs, Wc, bc):
    deg = np.bincount(np.asarray(dst).astype(np.int64), minlength=N)
    norm = np.where(deg < 1, 1.0, deg).astype(np.float32) ** -0.5
    gs, cells, NCH, TOT, R = _prep_edges(src, dst)

    x = np.asarray(x, np.float32)
    graph_ids = np.asarray(graph_ids, np.int64)
    wpk = np.zeros(((HOPS + 1) * F, NW), np.float16)
    for l in range(HLAYERS + 1):
        wpk[:, l * F : (l + 1) * F] = np.asarray(Ws[l], np.float32)
    wpk[0:F, (HLAYERS + 1) * F :] = np.asarray(Wc, np.float32)
    bcol = np.stack([np.asarray(b, np.float32) for b in bs], 1)
    bcr = np.tile(np.asarray(bc, np.float32)[None, :], (128, 1))

    in_maps = []
    for c in range(NCORES):
        xl = np.zeros((NPAD, F), np.float16)
        xl[:PER] = x[c * PER : (c + 1) * PER]
        big = xl.reshape(GRP, 128, F).transpose(1, 0, 2).reshape(128, GRP * F)
        nrm = np.ones(NPAD, np.float32)
        nrm[:PER] = norm[c * PER : (c + 1) * PER]
        gsl = np.full(NPAD, -1.0, np.float32)
        gsl[:PER] = graph_ids[c * PER : (c + 1) * PER]
        colf = np.zeros((128, 2 * GRP + 3 + CLASSES), np.float32)
        colf[:, 0:GRP] = nrm.reshape(GRP, 128).T
        colf[:, GRP : 2 * GRP] = gsl.reshape(GRP, 128).T
        colf[:, 2 * GRP : 2 * GRP + 3] = bcol
        colf[:, 2 * GRP + 3 :] = bcr
        colf16 = colf.astype(np.float16)
        wpk2 = wpk.reshape(HOPS + 1, 128, NW).transpose(1, 0, 2).reshape(
            128, (HOPS + 1) * NW)
        bigall = np.ascontiguousarray(
            np.concatenate([big, colf16, wpk2], axis=1))
        in_maps.append(dict(big_h=bigall, gs_idx=gs[c]))
    return in_maps, cells, NCH, TOT


def kernel(x, src, dst, graph_ids, W0, b0, W1, b1, W2, b2, Wc, bc, **_):
    in_maps, cells, NCH, TOT = _make_in_maps(
        x, src, dst, graph_ids, [W0, W1, W2], [b0, b1, b2], Wc, bc)
    nc = _build_program(cells, NCH, TOT)
    res = bass_utils.run_bass_kernel_spmd(nc, in_maps, core_ids=list(range(NCORES)))
    return np.asarray(res.results[0]["out"], np.float32)
